# revision 1
# baseline (speedup 1.0000x reference)
"""Trainium2 Bass kernel for nn_CrossAttention_9174050144362.

Reference computation (per batch b, spatial flattened to hw=4096):
    Q = Wq @ a + bq      [128, 4096]
    K = Wk @ p + bk      [128, 4096]
    V = Wv @ p + bv      [256, 4096]
    attn = softmax_n(Q^T K)            [4096, 4096]
    out  = V @ attn^T + a              [256, 4096]

Sharding: 8 cores = (4 batches) x (2 query halves of 2048). Each core
computes full K/V for its batch (recomputed per half; ~6% extra flops)
and attends its 2048 queries against all 4096 keys. No collectives.

Per-core kernel strategy (compute-bound, PE-centric):
  * All matmuls in float32r (FP22 truncated fp32) -> 1 cycle/row on PE.
  * S is computed TRANSPOSED: S^T[n, m] tiles via matmul(lhsT=K_tile,
    rhs=Q_chunk), so the softmax reduction over n is a partition-dim
    reduction. P^T = exp(S^T) goes straight from PSUM through ACT to
    SBUF, and feeds matmul(lhsT=V^T_tile, rhs=P^T) accumulation - no
    transposes of the big 4096x2048 matrix anywhere.
  * The softmax denominator is obtained by accumulating P^T tiles on
    the (otherwise idle) Vector engine, then one matmul with an
    all-ones [128,128] lhsT which simultaneously reduces over the
    partition dim and broadcasts the row-sums to all 128 partitions.
  * bv is folded into the epilogue: sum_n attn = 1, so V-bias adds
    bv[c] to every output pixel (a must stay pristine for the Q proj).
"""

import numpy as np

import concourse.bass as bass
import concourse.tile as tile
from concourse import bacc, mybir
from concourse.bass_utils import run_bass_kernel_spmd

B, C, H, W = 4, 256, 64, 64
HW = H * W            # 4096 keys
CH = C // 2           # 128 q/k channels
P = 128               # partitions
MS = HW // 2          # 2048 queries per core
MCH = 512             # query chunk (PSUM-bank sized)
NT = HW // P          # 32 key tiles
NCORES = 8

F32 = mybir.dt.float32
F32R = mybir.dt.float32r
BF16 = mybir.dt.bfloat16
AF = mybir.ActivationFunctionType

# Module-level knobs for the dev harness (test.py); harmless defaults for
# the grading path which just calls kernel(**inputs).
TRACE = False
TMPDIR = None
LAST_RESULT = None

_PROG = None
_ONES = np.ones((P, P), dtype=np.float32)


def _emit(tc, out_d, a_d, p_d, wqt_d, wkt_d, wvt_d, bq_d, bk_d, bv_d, ones_d):
    nc = tc.nc
    ts = bass.ts

    with (
        tc.tile_pool(name="statics", bufs=1) as statics,
        tc.tile_pool(name="ptp", bufs=6) as ptp,
        tc.tile_pool(name="accp", bufs=2) as accp,
        tc.tile_pool(name="rcp", bufs=2) as rcp,
        tc.tile_pool(name="osb", bufs=3) as osb,
        tc.tile_pool(name="psA", bufs=3, space="PSUM") as psA,
        tc.tile_pool(name="psOut", bufs=2, space="PSUM") as psOut,
        tc.tile_pool(name="psDen", bufs=1, space="PSUM") as psDen,
    ):
        # ---- statics into SBUF (c split as c = co*128 + ci, ci on partitions)
        # a/p are DMA'd in m-chunks so projections can start before the full
        # tensor lands (Tile tracks subtile deps).
        # small weight/bias DMAs FIRST so projections unblock within ~1us;
        # the bulk a/p loads stream behind them in m-chunks (subtile deps).
        wqt_sb = statics.tile([P, 2, CH], F32R)
        nc.sync.dma_start(wqt_sb[:], wqt_d.rearrange("(co ci) o -> ci co o", ci=P))
        wkt_sb = statics.tile([P, 2, CH], F32R)
        nc.sync.dma_start(wkt_sb[:], wkt_d.rearrange("(co ci) o -> ci co o", ci=P))
        wvt_sb = statics.tile([P, 2, C], F32R)
        nc.sync.dma_start(wvt_sb[:], wvt_d.rearrange("(co ci) o -> ci co o", ci=P))
        bq_sb = statics.tile([P, 1], F32)
        nc.sync.dma_start(bq_sb[:], bq_d[:])
        bk_sb = statics.tile([P, 1], F32)
        nc.sync.dma_start(bk_sb[:], bk_d[:])
        bv_sb = statics.tile([P, 2], F32)
        nc.sync.dma_start(bv_sb[:], bv_d[:])
        ones_sb = statics.tile([P, P], F32R)
        nc.sync.dma_start(ones_sb[:], ones_d[:])
        a_v = a_d.rearrange("(co ci) m -> ci co m", ci=P)
        a_sb = statics.tile([P, 2, MS], F32R)
        for h in range(4):
            nc.scalar.dma_start(a_sb[:, :, ts(h, MS // 4)], a_v[:, :, ts(h, MS // 4)])
        p_v = p_d.rearrange("(co ci) m -> ci co m", ci=P)
        p_sb = statics.tile([P, 2, HW], F32R)
        for h in range(8):
            nc.gpsimd.dma_start(p_sb[:, :, ts(h, HW // 8)], p_v[:, :, ts(h, HW // 8)])

        # ---- projections
        q_sb = statics.tile([P, MS], F32R)
        k_sb = statics.tile([P, HW], F32R)
        vt_sb = statics.tile([P, NT, C], BF16)

        def proj_q(t):
            ps_q = psA.tile([P, MCH], F32, tag="ps", name="ps_q")
            for co in range(2):
                nc.tensor.matmul(
                    ps_q[:], wqt_sb[:, co, :], a_sb[:, co, ts(t, MCH)],
                    start=(co == 0), stop=(co == 1),
                )
            nc.scalar.activation(
                q_sb[:, ts(t, MCH)], ps_q[:], AF.Identity, bias=bq_sb[:, 0:1]
            )

        def proj_k(t):
            ps_k = psA.tile([P, MCH], F32, tag="ps", name="ps_k")
            for co in range(2):
                nc.tensor.matmul(
                    ps_k[:], wkt_sb[:, co, :], p_sb[:, co, ts(t, MCH)],
                    start=(co == 0), stop=(co == 1),
                )
            nc.scalar.activation(
                k_sb[:, ts(t, MCH)], ps_k[:], AF.Identity, bias=bk_sb[:, 0:1]
            )

        def proj_vt(t):
            # V^T tile: vt[n, c] = sum_ci p[ci, n] * WvT[ci, c]  (no bias)
            ps_v = psA.tile([P, C], F32, tag="ps", name="ps_v")
            for co in range(2):
                nc.tensor.matmul(
                    ps_v[:], p_sb[:, co, ts(t, P)], wvt_sb[:, co, :],
                    start=(co == 0), stop=(co == 1),
                )
            nc.scalar.copy(vt_sb[:, t, :], ps_v[:])

        # emission interleaved to match DMA arrival: a quarters land on the
        # ACT queue while p eighths land on the GpSimd queue in parallel,
        # so alternate Q (needs a) with K/V^T (need p) instead of draining
        # each projection type serially.
        for h in range(4):
            proj_q(h)
            proj_k(2 * h)
            proj_k(2 * h + 1)
            for v in range(8 * h, 8 * h + 8):
                proj_vt(v)

        out_v = out_d.rearrange("(co ci) m -> ci co m", ci=P)

        def epilogue_den(acc, acc2):
            # denominator: ones^T @ (acc + acc2) reduces over n AND
            # broadcasts to all partitions (two accumulating matmuls)
            den_ps = psDen.tile([P, MCH], F32, tag="den")
            nc.tensor.matmul(den_ps[:], ones_sb[:], acc[:], start=True, stop=False)
            nc.tensor.matmul(den_ps[:], ones_sb[:], acc2[:], start=False, stop=True)
            recip = rcp.tile([P, MCH], F32, tag="rc")
            nc.vector.reciprocal(recip[:], den_ps[:])
            return recip

        def epilogue_out(mc, out_ps, recip):
            for co in range(2):
                o_sb = osb.tile([P, MCH], F32, tag="osb")
                nc.vector.tensor_mul(o_sb[:], out_ps[:, co, :], recip[:])
                nc.vector.tensor_add(o_sb[:], o_sb[:], a_sb[:, co, ts(mc, MCH)])
                # bv folded here: attn rows sum to 1, so V-bias is +bv[c]
                nc.vector.tensor_scalar_add(o_sb[:], o_sb[:], bv_sb[:, co : co + 1])
                nc.sync.dma_start(out_v[:, co, ts(mc, MCH)], o_sb[:])

        # ---- attention main loop over query chunks. Chunk k's
        # denominator matmul+reciprocal run early in chunk k+1 (t==2), its
        # normalize/store runs at t==6, so the PE stream never stalls on
        # the DVE accumulation chain; the final chunk pays a short tail.
        pending_den = None
        pending_out = None
        for mc in range(MS // MCH):
            out_ps = psOut.tile([P, 2, MCH], F32, tag="out")
            acc = accp.tile([P, MCH], F32R, tag="acc")
            acc2 = accp.tile([P, MCH], F32R, tag="acc2")
            prev_pt = None
            for t in range(NT):
                s_ps = psA.tile([P, MCH], F32, tag="ps")
                nc.tensor.matmul(
                    s_ps[:], k_sb[:, ts(t, P)], q_sb[:, ts(mc, MCH)],
                    start=True, stop=True,
                )
                pt = ptp.tile([P, MCH], BF16, tag="pt")
                nc.scalar.activation(pt[:], s_ps[:], AF.Exp)
                if t == 0:
                    nc.vector.tensor_copy(acc[:], pt[:])
                elif t == 1:
                    nc.gpsimd.tensor_copy(acc2[:], pt[:])
                elif t % 2 == 0:
                    nc.vector.tensor_add(acc[:], acc[:], pt[:])
                else:
                    nc.gpsimd.tensor_add(acc2[:], acc2[:], pt[:])
                # V.P matmuls run one iteration behind the S matmul so the
                # PE never waits on the exp of the tile it just produced.
                if prev_pt is not None:
                    tp, pt_prev = prev_pt
                    for co in range(2):
                        nc.tensor.matmul(
                            out_ps[:, co, :], vt_sb[:, tp, ts(co, P)], pt_prev[:],
                            start=(tp == 0), stop=False,
                        )
                prev_pt = (t, pt)
                if t == 2 and pending_den is not None:
                    pmc, pout, pacc, pacc2 = pending_den
                    pending_out = (pmc, pout, epilogue_den(pacc, pacc2))
                    pending_den = None
                if t == 6 and pending_out is not None:
                    epilogue_out(*pending_out)
                    pending_out = None
            tp, pt_prev = prev_pt
            for co in range(2):
                nc.tensor.matmul(
                    out_ps[:, co, :], vt_sb[:, tp, ts(co, P)], pt_prev[:],
                    start=False, stop=True,
                )
            pending_den = (mc, out_ps, acc, acc2)
        pmc, pout, pacc, pacc2 = pending_den
        epilogue_out(pmc, pout, epilogue_den(pacc, pacc2))


def _build():
    nc = bacc.Bacc("TRN2", target_bir_lowering=False, debug=False)
    a_d = nc.dram_tensor("a_s", [C, MS], F32R, kind="ExternalInput").ap()
    p_d = nc.dram_tensor("p_s", [C, HW], F32R, kind="ExternalInput").ap()
    wqt_d = nc.dram_tensor("wqt", [C, CH], F32R, kind="ExternalInput").ap()
    wkt_d = nc.dram_tensor("wkt", [C, CH], F32R, kind="ExternalInput").ap()
    wvt_d = nc.dram_tensor("wvt", [C, C], F32R, kind="ExternalInput").ap()
    bq_d = nc.dram_tensor("bq", [CH, 1], F32, kind="ExternalInput").ap()
    bk_d = nc.dram_tensor("bk", [CH, 1], F32, kind="ExternalInput").ap()
    bv_d = nc.dram_tensor("bv", [P, 2], F32, kind="ExternalInput").ap()
    ones_d = nc.dram_tensor("onesm", [P, P], F32R, kind="ExternalInput").ap()
    out_d = nc.dram_tensor("out_s", [C, MS], F32, kind="ExternalOutput").ap()
    with tile.TileContext(nc) as tc:
        _emit(tc, out_d, a_d, p_d, wqt_d, wkt_d, wvt_d, bq_d, bk_d, bv_d, ones_d)
    nc.compile()
    return nc


def _get_prog():
    global _PROG
    if _PROG is None:
        _PROG = _build()
    return _PROG


def kernel(**inputs):
    a = np.ascontiguousarray(np.asarray(inputs["a"], dtype=np.float32)).reshape(
        B, C, HW
    )
    p = np.ascontiguousarray(np.asarray(inputs["p"], dtype=np.float32)).reshape(
        B, C, HW
    )
    wqt = np.ascontiguousarray(np.asarray(inputs["Wq"], dtype=np.float32).T)
    wkt = np.ascontiguousarray(np.asarray(inputs["Wk"], dtype=np.float32).T)
    wvt = np.ascontiguousarray(np.asarray(inputs["Wv"], dtype=np.float32).T)
    bq = np.ascontiguousarray(np.asarray(inputs["bq"], dtype=np.float32)).reshape(
        CH, 1
    )
    bk = np.ascontiguousarray(np.asarray(inputs["bk"], dtype=np.float32)).reshape(
        CH, 1
    )
    bv = np.ascontiguousarray(
        np.asarray(inputs["bv"], dtype=np.float32).reshape(2, P).T
    )

    nc = _get_prog()
    in_maps = []
    for core in range(NCORES):
        b, h = divmod(core, 2)
        in_maps.append(
            {
                "a_s": np.ascontiguousarray(a[b, :, h * MS : (h + 1) * MS]),
                "p_s": p[b],
                "wqt": wqt,
                "wkt": wkt,
                "wvt": wvt,
                "bq": bq,
                "bk": bk,
                "bv": bv,
                "onesm": _ONES,
            }
        )
    kwargs = {}
    if TRACE:
        kwargs["trace"] = True
        if TMPDIR:
            kwargs["tmpdir"] = TMPDIR
    res = run_bass_kernel_spmd(nc, in_maps, core_ids=list(range(NCORES)), **kwargs)
    global LAST_RESULT
    LAST_RESULT = res

    out = np.empty((B, C, HW), dtype=np.float32)
    for core in range(NCORES):
        b, h = divmod(core, 2)
        out[b, :, h * MS : (h + 1) * MS] = res.results[core]["out_s"]
    return out.reshape(B, C, H, W)



# revision 4
# speedup vs baseline: 1.1638x; 1.1638x over previous
"""Trainium2 Bass kernel for nn_CrossAttention_9174050144362.

Reference computation (per batch b, spatial flattened to hw=4096):
    Q = Wq @ a + bq      [128, 4096]
    K = Wk @ p + bk      [128, 4096]
    V = Wv @ p + bv      [256, 4096]
    attn = softmax_n(Q^T K)            [4096, 4096]
    out  = V @ attn^T + a              [256, 4096]

Sharding: 8 cores = (4 batches) x (2 query halves of 2048). Each core
computes full K/V for its batch and attends its 2048 queries against all
4096 keys. No collectives.

v2 schedule (vs v1):
  * Flash-style prefix: chunk-0 S/exp/VP runs per p-eighth as the DMA
    lands, so the 15us input stream and the projections overlap the
    first attention chunk (PE warms early, no dead prefix).
  * exp outputs land in a persistent per-chunk pt store [32 tiles]; the
    softmax denominator is built by an in-place bf16 pair/quad/tree
    reduction (wide strided DVE ops) instead of 32 narrow accumulate
    adds racing GpSimd for the shared SBUF port.
  * partition-reduce + broadcast of the denominator in one ones-matmul;
    1/x via reciprocal_approx_fast (single DVE op, not iterative divide).
  * epilogue of chunk c (den matmul, recip, normalize, +a+bv, DMA out)
    is threaded through chunk c+1's pipeline. PSUM: 3 S banks + 2x2 out
    banks + 1 den bank = 8.
"""

import numpy as np

import concourse.bass as bass
import concourse.tile as tile
from concourse import bacc, mybir
from concourse.bass_utils import run_bass_kernel_spmd

B, C, H, W = 4, 256, 64, 64
HW = H * W            # 4096 keys
CH = C // 2           # 128 q/k channels
P = 128               # partitions
MS = HW // 2          # 2048 queries per core
MCH = 512             # query chunk (PSUM-bank sized)
NT = HW // P          # 32 key tiles
NCH = MS // MCH       # 4 query chunks
NCORES = 8

F32 = mybir.dt.float32
F32R = mybir.dt.float32r
BF16 = mybir.dt.bfloat16
AF = mybir.ActivationFunctionType

# Module-level knobs for the dev harness (test.py); harmless defaults for
# the grading path which just calls kernel(**inputs).
TRACE = False
TMPDIR = None
LAST_RESULT = None

_PROG = None
_ONES = np.ones((P, P), dtype=np.float32)


def _emit(tc, out_d, a_d, p_d, wqt_d, wkt_d, wvt_d, bq_d, bk_d, bv_d, ones_d):
    nc = tc.nc
    ts = bass.ts

    with (
        tc.tile_pool(name="statics", bufs=1) as statics,
        tc.tile_pool(name="rcp", bufs=2) as rcp,
        tc.tile_pool(name="osb", bufs=4) as osb,
        tc.tile_pool(name="psS", bufs=3, space="PSUM") as psS,
        tc.tile_pool(name="psOut", bufs=2, space="PSUM") as psOut,
        tc.tile_pool(name="psDen", bufs=1, space="PSUM") as psDen,
    ):
        # ---- small statics (sync queue, land first)
        wqt_sb = statics.tile([P, 2, CH], F32R)
        nc.sync.dma_start(wqt_sb[:], wqt_d.rearrange("(co ci) o -> ci co o", ci=P))
        wkt_sb = statics.tile([P, 2, CH], F32R)
        nc.sync.dma_start(wkt_sb[:], wkt_d.rearrange("(co ci) o -> ci co o", ci=P))
        wvt_sb = statics.tile([P, 2, C], F32R)
        nc.sync.dma_start(wvt_sb[:], wvt_d.rearrange("(co ci) o -> ci co o", ci=P))
        bq_sb = statics.tile([P, 1], F32)
        nc.sync.dma_start(bq_sb[:], bq_d[:])
        bk_sb = statics.tile([P, 1], F32)
        nc.sync.dma_start(bk_sb[:], bk_d[:])
        bv_sb = statics.tile([P, 2], F32)
        nc.sync.dma_start(bv_sb[:], bv_d[:])
        ones_sb = statics.tile([P, P], F32R)
        nc.sync.dma_start(ones_sb[:], ones_d[:])

        # ---- bulk inputs. scalar queue: a q0, then p odds / a rest
        # interleaved; gpsimd queue: p evens. Two rings share HBM so each
        # 512KB piece lands every ~2.2us alternating.
        a_v = a_d.rearrange("(co ci) m -> ci co m", ci=P)
        a_sb = statics.tile([P, 2, MS], F32R)
        p_v = p_d.rearrange("(co ci) m -> ci co m", ci=P)
        p_sb = statics.tile([P, 2, HW], F32R)
        E8 = HW // 8
        nc.scalar.dma_start(a_sb[:, :, ts(0, MCH)], a_v[:, :, ts(0, MCH)])
        for e in (0, 2, 4, 6):
            nc.gpsimd.dma_start(p_sb[:, :, ts(e, E8)], p_v[:, :, ts(e, E8)])
        sc_order = [("p", 1), ("p", 3), ("a", 1), ("p", 5), ("a", 2), ("p", 7),
                    ("a", 3)]
        for kind, i in sc_order:
            if kind == "p":
                nc.scalar.dma_start(p_sb[:, :, ts(i, E8)], p_v[:, :, ts(i, E8)])
            else:
                nc.scalar.dma_start(a_sb[:, :, ts(i, MCH)], a_v[:, :, ts(i, MCH)])

        # ---- persistent SBUF state
        q_sb = statics.tile([P, MS], F32R)
        k_sb = statics.tile([P, HW], F32R)
        vt_sb = statics.tile([P, NT, C], BF16)
        # per-chunk exp store, 2 rotating slots; pair/quad/tree reductions
        # happen in place over tile slots.
        pt_sb = statics.tile([P, 2, NT, MCH], BF16)
        acc_sb = statics.tile([P, 2, MCH], F32R)   # final den accumulators
        ab2_sb = statics.tile([P, 2, MS], F32)     # a + bv (epilogue residual)

        out_v = out_d.rearrange("(co ci) m -> ci co m", ci=P)

        # ---------- helpers ----------
        def proj_q(c):
            ps = psS.tile([P, MCH], F32, tag="ps", name="ps_q")
            for co in range(2):
                nc.tensor.matmul(ps[:], wqt_sb[:, co, :], a_sb[:, co, ts(c, MCH)],
                                 start=(co == 0), stop=(co == 1))
            nc.scalar.activation(q_sb[:, ts(c, MCH)], ps[:], AF.Identity,
                                 bias=bq_sb[:, 0:1])

        def proj_k(e):
            ps = psS.tile([P, MCH], F32, tag="ps", name="ps_k")
            for co in range(2):
                nc.tensor.matmul(ps[:], wkt_sb[:, co, :], p_sb[:, co, ts(e, E8)],
                                 start=(co == 0), stop=(co == 1))
            nc.scalar.activation(k_sb[:, ts(e, E8)], ps[:], AF.Identity,
                                 bias=bk_sb[:, 0:1])

        def proj_vt(t):
            # V^T tile: vt[n, c] = sum_ci p[ci, n] * WvT[ci, c]  (no bias;
            # bv folded into ab2 since attn rows sum to 1)
            ps = psS.tile([P, C], F32, tag="ps", name="ps_v")
            for co in range(2):
                nc.tensor.matmul(ps[:], p_sb[:, co, ts(t, P)], wvt_sb[:, co, :],
                                 start=(co == 0), stop=(co == 1))
            # eviction alternates ACT/DVE to balance prefix engine load
            if t % 4 < 3:
                nc.vector.tensor_copy(vt_sb[:, t, :], ps[:])
            else:
                nc.scalar.copy(vt_sb[:, t, :], ps[:])

        def s_exp(c, t):
            s = c % 2
            ps = psS.tile([P, MCH], F32, tag="ps", name="ps_s")
            nc.tensor.matmul(ps[:], k_sb[:, ts(t, P)], q_sb[:, ts(c, MCH)],
                             start=True, stop=True)
            nc.scalar.activation(pt_sb[:, s, t, :], ps[:], AF.Exp)

        def vp(c, t, out_ps, last=False):
            s = c % 2
            for co in range(2):
                nc.tensor.matmul(out_ps[:, co, :],
                                 vt_sb[:, t, ts(co, P)], pt_sb[:, s, t, :],
                                 start=(t == 0), stop=last)

        def pair(c, j):
            # pt[2j] += pt[2j+1]  (in place, bf16 2x mode)
            s = c % 2
            nc.vector.tensor_add(pt_sb[:, s, 2 * j, :], pt_sb[:, s, 2 * j, :],
                                 pt_sb[:, s, 2 * j + 1, :])

        def quad(c, k):
            # pt[4k] += pt[4k+2]  (gpsimd; reads DVE pair outputs)
            s = c % 2
            nc.gpsimd.tensor_add(pt_sb[:, s, 4 * k, :], pt_sb[:, s, 4 * k, :],
                                 pt_sb[:, s, 4 * k + 2, :])

        def den_tree(c):
            # 8 quads at slots {0,4,...,28} -> acc_sb[c%2] via 3 wide strided
            # in-place adds (bf16) + final f32r output.
            s = c % 2
            v = pt_sb[:].rearrange("p s (t q) m -> p s t q m", q=4)
            nc.vector.tensor_add(v[:, s, 0:4, 0, :], v[:, s, 0:4, 0, :],
                                 v[:, s, 4:8, 0, :])
            nc.vector.tensor_add(v[:, s, 0:2, 0, :], v[:, s, 0:2, 0, :],
                                 v[:, s, 2:4, 0, :])
            nc.vector.tensor_add(acc_sb[:, s, :], v[:, s, 0, 0, :],
                                 v[:, s, 1, 0, :])

        def den_mm(c):
            # ones^T @ acc: reduces over partitions AND broadcasts row-sums
            den_ps = psDen.tile([P, MCH], F32, tag="den")
            nc.tensor.matmul(den_ps[:], ones_sb[:], acc_sb[:, c % 2, :],
                             start=True, stop=True)
            return den_ps

        def recip_of(den_ps):
            r = rcp.tile([P, MCH], F32, tag="rc")
            nc.vector.reciprocal_approx_fast(out=r[:], in_=den_ps[:])
            return r

        def ep_mul(c, out_ps, r, co):
            o = osb.tile([P, MCH], F32, tag="osb")
            nc.vector.tensor_mul(o[:], out_ps[:, co, :], r[:])
            return o

        def ep_add_dma(c, o, co):
            nc.vector.tensor_add(o[:], o[:], ab2_sb[:, co, ts(c, MCH)])
            nc.sync.dma_start(out_v[:, co, ts(c, MCH)], o[:])

        def ab2(c):
            for co in range(2):
                nc.vector.tensor_scalar_add(ab2_sb[:, co, ts(c, MCH)],
                                            a_sb[:, co, ts(c, MCH)],
                                            bv_sb[:, co:co + 1])

        # ---------- prefix: projections + chunk 0, flash-style ----------
        proj_q(0)
        prev_t = None
        out_ps0 = psOut.tile([P, 2, MCH], F32, tag="out")
        for e in range(8):
            proj_k(e)
            for t in range(4 * e, 4 * e + 4):
                proj_vt(t)
            if e in (2, 4, 6):
                proj_q(e // 2)
            for t in range(4 * e, 4 * e + 4):
                s_exp(0, t)
                if prev_t is not None:
                    vp(0, prev_t, out_ps0)
                prev_t = t
                # den partials for chunk 0 (pairs on DVE, quads on gpsimd)
                if t % 2 == 1:
                    pair(0, t // 2)
                if t % 4 == 3:
                    quad(0, t // 4)
        vp(0, 31, out_ps0, last=True)
        ab2(0)
        ab2(1)
        ab2(2)
        ab2(3)

        # ---------- steady chunks 1..3, carrying chunk c-1 epilogue ----
        carry = (0, out_ps0)  # chunk whose epilogue runs during this chunk
        for c in range(1, NCH):
            pc, pout = carry
            den_tree(pc)  # DVE, early in chunk (reads quads of pc)
            out_ps = psOut.tile([P, 2, MCH], F32, tag="out")
            prev_t = None
            pending = {}
            for t in range(NT):
                s_exp(c, t)
                if prev_t is not None:
                    vp(c, prev_t, out_ps)
                prev_t = t
                if t % 2 == 1:
                    pair(c, t // 2)
                if t % 4 == 3:
                    quad(c, t // 4)
                if t == 8:
                    pending["den"] = den_mm(pc)
                elif t == 10:
                    pending["rc"] = recip_of(pending.pop("den"))
                elif t == 12:
                    pending["o0"] = ep_mul(pc, pout, pending["rc"], 0)
                elif t == 14:
                    ep_add_dma(pc, pending.pop("o0"), 0)
                elif t == 16:
                    pending["o1"] = ep_mul(pc, pout, pending.pop("rc"), 1)
                elif t == 18:
                    ep_add_dma(pc, pending.pop("o1"), 1)
            vp(c, 31, out_ps, last=True)
            carry = (c, out_ps)

        # ---------- tail: chunk 3 epilogue ----------
        pc, pout = carry
        den_tree(pc)
        den_ps = den_mm(pc)
        r = recip_of(den_ps)
        for co in range(2):
            o = ep_mul(pc, pout, r, co)
            ep_add_dma(pc, o, co)


def _build():
    nc = bacc.Bacc("TRN2", target_bir_lowering=False, debug=False)
    a_d = nc.dram_tensor("a_s", [C, MS], F32R, kind="ExternalInput").ap()
    p_d = nc.dram_tensor("p_s", [C, HW], F32R, kind="ExternalInput").ap()
    wqt_d = nc.dram_tensor("wqt", [C, CH], F32R, kind="ExternalInput").ap()
    wkt_d = nc.dram_tensor("wkt", [C, CH], F32R, kind="ExternalInput").ap()
    wvt_d = nc.dram_tensor("wvt", [C, C], F32R, kind="ExternalInput").ap()
    bq_d = nc.dram_tensor("bq", [CH, 1], F32, kind="ExternalInput").ap()
    bk_d = nc.dram_tensor("bk", [CH, 1], F32, kind="ExternalInput").ap()
    bv_d = nc.dram_tensor("bv", [P, 2], F32, kind="ExternalInput").ap()
    ones_d = nc.dram_tensor("onesm", [P, P], F32R, kind="ExternalInput").ap()
    out_d = nc.dram_tensor("out_s", [C, MS], F32, kind="ExternalOutput").ap()
    with tile.TileContext(nc) as tc:
        _emit(tc, out_d, a_d, p_d, wqt_d, wkt_d, wvt_d, bq_d, bk_d, bv_d, ones_d)
    nc.compile()
    return nc


def _get_prog():
    global _PROG
    if _PROG is None:
        _PROG = _build()
    return _PROG


def kernel(**inputs):
    a = np.ascontiguousarray(np.asarray(inputs["a"], dtype=np.float32)).reshape(
        B, C, HW
    )
    p = np.ascontiguousarray(np.asarray(inputs["p"], dtype=np.float32)).reshape(
        B, C, HW
    )
    wqt = np.ascontiguousarray(np.asarray(inputs["Wq"], dtype=np.float32).T)
    wkt = np.ascontiguousarray(np.asarray(inputs["Wk"], dtype=np.float32).T)
    wvt = np.ascontiguousarray(np.asarray(inputs["Wv"], dtype=np.float32).T)
    bq = np.ascontiguousarray(np.asarray(inputs["bq"], dtype=np.float32)).reshape(
        CH, 1
    )
    bk = np.ascontiguousarray(np.asarray(inputs["bk"], dtype=np.float32)).reshape(
        CH, 1
    )
    bv = np.ascontiguousarray(
        np.asarray(inputs["bv"], dtype=np.float32).reshape(2, P).T
    )

    nc = _get_prog()
    in_maps = []
    for core in range(NCORES):
        b, h = divmod(core, 2)
        in_maps.append(
            {
                "a_s": np.ascontiguousarray(a[b, :, h * MS : (h + 1) * MS]),
                "p_s": p[b],
                "wqt": wqt,
                "wkt": wkt,
                "wvt": wvt,
                "bq": bq,
                "bk": bk,
                "bv": bv,
                "onesm": _ONES,
            }
        )
    kwargs = {}
    if TRACE:
        kwargs["trace"] = True
        if TMPDIR:
            kwargs["tmpdir"] = TMPDIR
    res = run_bass_kernel_spmd(nc, in_maps, core_ids=list(range(NCORES)), **kwargs)
    global LAST_RESULT
    LAST_RESULT = res

    out = np.empty((B, C, HW), dtype=np.float32)
    for core in range(NCORES):
        b, h = divmod(core, 2)
        out[b, :, h * MS : (h + 1) * MS] = res.results[core]["out_s"]
    return out.reshape(B, C, H, W)
